# revision 1
# baseline (speedup 1.0000x reference)
"""Multi-head attention forward for TRN2, 8 NeuronCores, data-parallel over batch.

Reference computation (B=16, S=1024, D=768, H=12, HD=64), fp32:
    q = einsum('bsd,dhe->bshe', x, Wq) + bq        (same for k, v)
    z = einsum('bqhd,bkhd->bhqk', q/8, k)
    a = softmax(z, axis=-1)
    o = einsum('bhqk,bkhd->bqhd', a, v)
    y = einsum('bqhd,hde->bqe', o, Wo) + bo

Design (per core, 2 batches, phases pipelined by the Tile scheduler):
  - One orientation flip at input: x [S,D] -> xT [D,S] via PE transpose
    (fp32 is_transpose matmuls, 6 per seq tile into one 2-bank PSUM tile).
  - Projections produce QT,KT [D,S] (head-transposed) and V [S,D] directly
    from xT; all feed-forward tensors are written as float32r by the
    evicting engine (BIR requires fp32r matmul inputs to be rounded by
    their producer).
  - Scores computed transposed: zT[k,q] = KT_slice.T @ QT_slice
    (contraction=64). Heads are processed in pairs: the even/odd head's
    score matmuls sit at PE row groups 0-63/64-127 (tile_position derived
    from base_partition) as adjacent instructions, so the hardware runs
    them concurrently (row-tiling).
  - exp on ACT with scale=1/8 fused; no max-subtraction needed (|z|<~3).
  - PV: U_ext[0:65,q] = sum_k Vext[k,0:65].T @ expZT[k,q]; Vext carries a
    ones column so the softmax denominator accumulates in PSUM row 64.
  - U+denom evicted to SBUF immediately (frees the PSUM accumulator), then:
    DMA partition-broadcast of the denominator row (gpsimd SWDGE queue),
    reciprocal_approx_fast on DVE, DVE tensor_mul -> OTn [D,S], which is
    exactly the out-projection stationary layout. Odd heads are
    DMA-shifted to partitions 64-127 (DVE lanes cannot cross partitions).
  - y[q,d] = sum_c OTn[c,q-128].T @ Wo[c,d] (+ ones x cvec rank-1 when
    biases are nonzero; cvec = bv@Wo + bo; bq/bk fold into the QT/KT
    evictions; bv/bo commute through softmax normalization exactly).
  - All matmuls in float32r (tf32-like, ~1.4e-4 rel err, 1 cycle/row at
    N>=256 vs 4 for fp32). End-to-end rel err vs fp32 reference: 2.6e-4.
  - Big DMAs batched as [128, 2, 768] tile pairs on the sync queue;
    phase-C DMAs ride the gpsimd queue to keep the sync queue clear; the
    out-projection weight prefetches during attention.
  - TimelineSim cost model: 412 us per core (PE work 378 us). Fusing the
    projections into the attention loop to fill PE during the ACT-bound
    attention was tried and reverted: the projection PSUM accumulators
    contend with the score tiles for the two spare PSUM slots (8 banks
    total: scores 2x2 + PV accumulators 2x2), which stalls the exp
    pipeline and costs more than the overlap wins.
"""

import numpy as np
from contextlib import ExitStack

import concourse.bacc as bacc
import concourse.bass as bass
import concourse.tile as tile
import concourse.mybir as mybir
from concourse.bass_utils import run_bass_kernel_spmd
from concourse.masks import make_identity

B, S, D, H, HD = 16, 1024, 768, 12, 64
NCORES = 8
BL = B // NCORES      # batches per core
P = 128
DC = D // P           # 6 contraction chunks
SQ = S // P           # 8 seq tiles of 128
F32 = mybir.dt.float32
F32R = mybir.dt.float32r
EXP = mybir.ActivationFunctionType.Exp
SCALE = 1.0 / float(np.sqrt(HD))

_NC = {}
_DEBUG = False  # add DRAM dumps of intermediates (batch 0)


def _emit(tc, x_d, w_d, b_d, y_d, dbg=None, with_bias=True):
    """Emit the whole per-core program. w_d/b_d: dicts of DRAM APs."""
    nc = tc.nc

    def dump(name, sbuf_ap):
        if dbg is not None and name in dbg:
            nc.sync.dma_start(out=dbg[name], in_=sbuf_ap)

    with ExitStack() as ctx:
        consts = ctx.enter_context(tc.tile_pool(name="consts", bufs=1))
        wpool = ctx.enter_context(tc.tile_pool(name="wpool", bufs=2))
        big = ctx.enter_context(tc.tile_pool(name="big", bufs=1))
        atp = ctx.enter_context(
            tc.tile_pool(name="atp", bufs=(2 if with_bias else 3)))
        iop = ctx.enter_context(tc.tile_pool(name="iop", bufs=3))
        smal = ctx.enter_context(tc.tile_pool(name="smal", bufs=2))
        pp = ctx.enter_context(tc.tile_pool(name="pp", bufs=2, space="PSUM"))

        # ---- constants ----
        ident = consts.tile([P, P], F32)
        make_identity(nc, ident)
        if with_bias:
            bq_sb = consts.tile([P, DC], F32)
            nc.sync.dma_start(out=bq_sb,
                              in_=b_d["bq"].rearrange("(c p) -> p c", p=P))
            bk_sb = consts.tile([P, DC], F32)
            nc.sync.dma_start(out=bk_sb,
                              in_=b_d["bk"].rearrange("(c p) -> p c", p=P))
            bv_st = consts.tile([P, DC], F32)
            nc.sync.dma_start(out=bv_st,
                              in_=b_d["bv"].rearrange("(c p) -> p c", p=P))
            bv_r = consts.tile([P, DC], F32R)
            nc.vector.tensor_copy(bv_r, bv_st)
            bo_st = consts.tile([1, D], F32)
            nc.sync.dma_start(out=bo_st, in_=b_d["bo"].unsqueeze(0))
            bo_r = consts.tile([1, D], F32R)
            nc.vector.tensor_copy(bo_r, bo_st)
            ones_f32 = consts.tile([1, P], F32)
            nc.vector.memset(ones_f32, 1.0)
            ones_row_r = consts.tile([1, P], F32R)
            nc.vector.tensor_copy(ones_row_r, ones_f32)
            cvec_sb = consts.tile([1, D], F32R)
        ones96 = consts.tile([P, SQ * H], F32)
        nc.vector.memset(ones96, 1.0)
        # warm the ACT exp table at t=0 so the ~2.7us table load overlaps
        # phase A instead of stalling the first attention tile
        expwarm = consts.tile([1, 1], F32)
        nc.scalar.activation(expwarm, ones96[0:1, 0:1], EXP)
        cvec_done = False

        def load_weight(name):
            # DMA fp32 2-chunk pairs through staging, round to f32r on gpsimd
            wr = wpool.tile([P, DC, D], F32R, tag="w", name=f"w_{name}")
            src = w_d[name].rearrange("(c p) m -> p c m", p=P)
            for c in range(0, DC, 2):
                ws = iop.tile([P, 2, D], F32, tag="st2", name=f"ws_{name}_{c}")
                nc.sync.dma_start(out=ws, in_=src[:, c:c + 2, :])
                nc.vector.tensor_copy(wr[:, c:c + 2, :], ws)
            return wr

        for b in range(BL):
            x_b = x_d[b].rearrange("(t p) d -> p t d", p=P)
            y_b = y_d[b].rearrange("(t p) d -> p t d", p=P)

            # ---- phase A: x -> xT (f32r) ----
            xT = big.tile([P, DC, S], F32R, tag="xT", name=f"xT_{b}")
            for sq in range(0, SQ, 2):
                x_in = iop.tile([P, 2, D], F32, tag="st2", name=f"xin_{b}_{sq}")
                if b == 0 and sq == 0:
                    # split the first load so the first transposes start as
                    # soon as the first columns land (shorter kernel lead-in)
                    nc.sync.dma_start(out=x_in[:, 0, 0:384],
                                      in_=x_b[:, sq, 0:384])
                    nc.sync.dma_start(out=x_in[:, 0, 384:D],
                                      in_=x_b[:, sq, 384:D])
                    nc.sync.dma_start(out=x_in[:, 1, :], in_=x_b[:, sq + 1, :])
                else:
                    nc.sync.dma_start(out=x_in, in_=x_b[:, sq:sq + 2, :])
                for j in range(2):
                    tt = pp.tile([P, 1024], F32, tag="mm",
                                 name=f"tps_{b}_{sq}_{j}")
                    for c in range(DC):
                        nc.tensor.transpose(
                            tt[:, c * P:(c + 1) * P],
                            x_in[:, j, c * P:(c + 1) * P], ident
                        )
                    nc.vector.tensor_copy(
                        xT[:, :, (sq + j) * P:(sq + j + 1) * P],
                        tt[:, :D].rearrange("p (c q) -> p c q", c=DC),
                    )

            if b == 0:
                dump("xT", xT)

            # ---- phase B: projections ----
            wq_r = load_weight("wq")
            QT = big.tile([P, DC, S], F32R, tag="QT", name=f"QT_{b}")
            for m in range(DC):
                qq = pp.tile([P, 1024], F32, tag="mm", name=f"qps_{b}_{m}")
                for c in range(DC):
                    for hf in range(2):
                        nc.tensor.matmul(
                            qq[:, hf * 512:(hf + 1) * 512],
                            wq_r[:, c, m * P:(m + 1) * P],
                            xT[:, c, hf * 512:(hf + 1) * 512],
                            start=(c == 0), stop=(c == DC - 1),
                        )
                if with_bias:
                    nc.vector.tensor_scalar_add(QT[:, m, :], qq,
                                                bq_sb[:, m:m + 1])
                else:
                    nc.vector.tensor_copy(QT[:, m, :], qq)

            wk_r = load_weight("wk")
            KT = big.tile([P, DC, S], F32R, tag="KT", name=f"KT_{b}")
            for m in range(DC):
                kk = pp.tile([P, 1024], F32, tag="mm", name=f"kps_{b}_{m}")
                for c in range(DC):
                    for hf in range(2):
                        nc.tensor.matmul(
                            kk[:, hf * 512:(hf + 1) * 512],
                            wk_r[:, c, m * P:(m + 1) * P],
                            xT[:, c, hf * 512:(hf + 1) * 512],
                            start=(c == 0), stop=(c == DC - 1),
                        )
                if with_bias:
                    nc.vector.tensor_scalar_add(KT[:, m, :], kk,
                                                bk_sb[:, m:m + 1])
                else:
                    nc.vector.tensor_copy(KT[:, m, :], kk)

            wv_r = load_weight("wv")
            # V layout [P, SQ, H, 65]: cols 0..63 = v, col 64 = ones
            V = big.tile([P, SQ, H, 65], F32R, tag="V", name=f"V_{b}")
            nc.vector.tensor_copy(
                V[:, :, :, 64], ones96.rearrange("p (a h) -> p a h", a=SQ)
            )
            for sq in range(SQ):
                vv = pp.tile([P, 1024], F32, tag="mm", name=f"vps_{b}_{sq}")
                for c in range(DC):
                    nc.tensor.matmul(
                        vv[:, 0:512], xT[:, c, sq * P:(sq + 1) * P],
                        wv_r[:, c, 0:512], start=(c == 0), stop=(c == DC - 1),
                    )
                    nc.tensor.matmul(
                        vv[:, 512:D], xT[:, c, sq * P:(sq + 1) * P],
                        wv_r[:, c, 512:D], start=(c == 0), stop=(c == DC - 1),
                    )
                nc.vector.tensor_copy(
                    V[:, sq, :, 0:64],
                    vv[:, :D].rearrange("p (h e) -> p h e", h=H),
                )
            if b == 0:
                dump("QT", QT)
                dump("KT", KT)
                dump("V", V)

            # prefetch output-projection weight during attention
            wo_r = load_weight("wo")
            if with_bias and not cvec_done:
                cvec_done = True
                cv = pp.tile([P, 1024], F32, tag="ov", name="cvps")
                for c in range(DC):
                    nc.tensor.matmul(cv[0:1, 0:512], bv_r[:, c:c + 1],
                                     wo_r[:, c, 0:512], start=(c == 0),
                                     stop=False)
                    nc.tensor.matmul(cv[0:1, 512:D], bv_r[:, c:c + 1],
                                     wo_r[:, c, 512:D], start=(c == 0),
                                     stop=False)
                nc.tensor.matmul(cv[0:1, 0:512], ones_row_r[:, 0:1],
                                 bo_r[:, 0:512], start=False, stop=True)
                nc.tensor.matmul(cv[0:1, 512:D], ones_row_r[:, 0:1],
                                 bo_r[:, 512:D], start=False, stop=True)
                nc.vector.tensor_copy(cvec_sb, cv[0:1, :D])

            # ---- phase C: attention, head pairs interleaved ----
            OTn = big.tile([P, DC, S], F32R, tag="OTn", name=f"OTn_{b}")
            for ch in range(DC):
                oos = [pp.tile([P, 1024], F32, tag="ov",
                               name=f"ops_{b}_{ch}_{par}")
                       for par in range(2)]
                for kt in range(SQ):
                    # issue order e0,o0,e1,o1: each q-half's even/odd matmuls
                    # are adjacent and row-group-disjoint -> PE runs both
                    # concurrently (row tiling)
                    zzs = [pp.tile([P, 1024], F32, tag="mm",
                                   name=f"zps_{b}_{ch}_{par}_{kt}")
                           for par in range(2)]
                    for hf in range(2):
                        for par in range(2):
                            psl = slice(par * 64, par * 64 + 64)
                            ksl = KT[psl, ch, kt * P:(kt + 1) * P]
                            nc.tensor.matmul(
                                zzs[par][:, hf * 512:(hf + 1) * 512], ksl,
                                QT[psl, ch, hf * 512:(hf + 1) * 512],
                                start=True, stop=True)
                    ats = []
                    for par in range(2):
                        at = atp.tile([P, 1024], F32R, tag="at",
                                      name=f"at_{b}_{ch}_{par}_{kt}")
                        nc.scalar.activation(at, zzs[par], EXP, scale=SCALE)
                        ats.append(at)
                        if b == 0 and ch == 0 and par == 0 and kt == 0:
                            dump("at0", at)
                    for par in range(2):
                        vsl = V[:, kt, 2 * ch + par, :]
                        for hf in range(2):
                            nc.tensor.matmul(
                                oos[par][0:65, hf * 512:(hf + 1) * 512],
                                vsl, ats[par][:, hf * 512:(hf + 1) * 512],
                                start=(kt == 0), stop=(kt == SQ - 1),
                            )
                # evict U+denom, then normalize from SBUF
                ues = []
                for par in range(2):
                    ue = smal.tile([65, S], F32, tag="ub",
                                   name=f"ue_{b}_{ch}_{par}",
                                   bufs=(1 if with_bias else 2))
                    nc.vector.tensor_copy(ue, oos[par][0:65, :])
                    ues.append(ue)
                for par in range(2):
                    h = 2 * ch + par
                    psl = slice(par * 64, par * 64 + 64)
                    ue = ues[par]
                    dsl = ue[64:65, :]
                    rbraw = smal.tile([64, S], F32, tag="rbraw",
                                      name=f"rbraw_{b}_{h}", bufs=1)
                    srcap = bass.AP(tensor=dsl.tensor, offset=dsl.offset,
                                    ap=[list(dsl.ap[0]), [0, 64],
                                        list(dsl.ap[1])])
                    nc.gpsimd.dma_start(out=rbraw, in_=srcap)
                    rb = smal.tile([64, S], F32, tag="rb",
                                   name=f"rb_{b}_{h}",
                                   bufs=(1 if with_bias else 2))
                    nc.vector.reciprocal_approx_fast(out=rb, in_=rbraw)
                    if par == 0:
                        nc.vector.tensor_mul(OTn[psl, ch, :], ue[0:64, :], rb)
                    else:
                        stg = smal.tile([64, S], F32R, tag="rbraw",
                                        name=f"stg_{b}_{h}", bufs=1)
                        nc.vector.tensor_mul(stg, ue[0:64, :], rb)
                        nc.gpsimd.dma_start(out=OTn[psl, ch, :], in_=stg)
                    if b == 0 and h == 0:
                        dump("rbi0", rb)
            if b == 0:
                dump("OTn", OTn)

            # ---- phase D/E: output projection ----
            for sq in range(0, SQ, 2):
                yst = iop.tile([P, 2, D], F32, tag="st2", name=f"yst_{b}_{sq}")
                # split the final store so its first half ships while the
                # last tile is still evicting (shorter kernel tail)
                split = (b == BL - 1 and sq == SQ - 2)
                for j in range(2):
                    yy = pp.tile([P, 1024], F32, tag="mm",
                                 name=f"yps_{b}_{sq}_{j}")
                    for c in range(DC):
                        st = OTn[:, c, (sq + j) * P:(sq + j + 1) * P]
                        last = (not with_bias) and c == DC - 1
                        nc.tensor.matmul(yy[:, 0:512], st, wo_r[:, c, 0:512],
                                         start=(c == 0), stop=last)
                        nc.tensor.matmul(yy[:, 512:D], st, wo_r[:, c, 512:D],
                                         start=(c == 0), stop=last)
                    if with_bias:
                        nc.tensor.matmul(yy[:, 0:512], ones_row_r,
                                         cvec_sb[:, 0:512], start=False,
                                         stop=True)
                        nc.tensor.matmul(yy[:, 512:D], ones_row_r,
                                         cvec_sb[:, 512:D], start=False,
                                         stop=True)
                    if split and j == 1:
                        # last tile: evict+store in halves so the store
                        # pipelines with the eviction (shorter tail)
                        nc.vector.tensor_copy(yst[:, j, 0:384], yy[:, 0:384])
                        nc.sync.dma_start(out=y_b[:, sq + j, 0:384],
                                          in_=yst[:, j, 0:384])
                        nc.vector.tensor_copy(yst[:, j, 384:D],
                                              yy[:, 384:D])
                        nc.sync.dma_start(out=y_b[:, sq + j, 384:D],
                                          in_=yst[:, j, 384:D])
                    else:
                        nc.vector.tensor_copy(yst[:, j, :], yy[:, :D])
                        if split:
                            nc.sync.dma_start(out=y_b[:, sq + j, :],
                                              in_=yst[:, j, :])
                if not split:
                    nc.sync.dma_start(out=y_b[:, sq:sq + 2, :], in_=yst)


def _build(with_bias=True):
    nc = bacc.Bacc("TRN2", target_bir_lowering=False, debug=False,
                   num_devices=NCORES)
    x_d = nc.dram_tensor("x", [BL, S, D], F32, kind="ExternalInput").ap()
    w_d = {n: nc.dram_tensor(n, [D, D], F32, kind="ExternalInput").ap()
           for n in ("wq", "wk", "wv", "wo")}
    b_d = {n: nc.dram_tensor(n, [D], F32, kind="ExternalInput").ap()
           for n in ("bq", "bk", "bv", "bo")}
    y_d = nc.dram_tensor("y", [BL, S, D], F32, kind="ExternalOutput").ap()
    dbg = None
    if _DEBUG:
        shapes = {"xT": ([P, DC, S], F32R), "QT": ([P, DC, S], F32R),
                  "KT": ([P, DC, S], F32R), "V": ([P, SQ, H, 65], F32R),
                  "at0": ([P, S], F32R), "rb0": ([65, S], F32),
                  "rbi0": ([64, S], F32),
                  "OTn": ([P, DC, S], F32R)}
        dbg = {n: nc.dram_tensor(f"dbg_{n}", sh, dt,
                                 kind="ExternalOutput").ap()
               for n, (sh, dt) in shapes.items()}
    with tile.TileContext(nc) as tc:
        _emit(tc, x_d, w_d, b_d, y_d, dbg, with_bias=with_bias)
    nc.compile()
    return nc


def _in_maps(x, Wq, bq, Wk, bk, Wv, bv, Wo, bo):
    # convert to host numpy before reshaping so jax-array inputs don't
    # trigger device-side ops
    def _np(a, shape):
        return np.ascontiguousarray(
            np.asarray(a, dtype=np.float32).reshape(shape))

    w = {
        "wq": _np(Wq, (D, D)), "wk": _np(Wk, (D, D)),
        "wv": _np(Wv, (D, D)), "wo": _np(Wo, (D, D)),
        "bq": _np(bq, (D,)), "bk": _np(bk, (D,)),
        "bv": _np(bv, (D,)), "bo": _np(bo, (D,)),
    }
    x = np.asarray(x, dtype=np.float32)
    return [dict(w, x=np.ascontiguousarray(x[i * BL:(i + 1) * BL]))
            for i in range(NCORES)]


def get_nc(with_bias=True):
    if with_bias not in _NC:
        _NC[with_bias] = _build(with_bias=with_bias)
    return _NC[with_bias]


def run(inputs, trace=False):
    with_bias = any(
        np.any(np.asarray(inputs[k])) for k in ("bq", "bk", "bv", "bo"))
    nc = get_nc(with_bias=with_bias)
    maps = _in_maps(**inputs)
    res = run_bass_kernel_spmd(nc, maps, list(range(NCORES)), trace=trace)
    y = np.concatenate([res.results[i]["y"] for i in range(NCORES)], axis=0)
    return y, res


def kernel(x, Wq, bq, Wk, bk, Wv, bv, Wo, bo):
    y, _ = run(dict(x=x, Wq=Wq, bq=bq, Wk=Wk, bk=bk, Wv=Wv, bv=bv,
                    Wo=Wo, bo=bo))
    return y



# revision 17
# speedup vs baseline: 1.1918x; 1.1918x over previous
"""Multi-head attention forward for TRN2, 8 NeuronCores, data-parallel over batch.

Reference computation (B=16, S=1024, D=768, H=12, HD=64), fp32:
    q = einsum('bsd,dhe->bshe', x, Wq) + bq        (same for k, v)
    z = einsum('bqhd,bkhd->bhqk', q/8, k)
    a = softmax(z, axis=-1)
    o = einsum('bhqk,bkhd->bqhd', a, v)
    y = einsum('bqhd,hde->bqe', o, Wo) + bo

Design (per core, 2 batches, phases pipelined by the Tile scheduler):
  - x [S,D] -> xT [D,S] via PE fp32 transposes, evicted to bf16.
  - Q/K/V projections in bf16 (natural layout, chunked contraction).
  - Q/K evictions scale by 64 and round to fp8e4 in the natural layout,
    then 24 small gpsimd DMAs shuffle them into the DoubleRow quad layout
    QT_dr/KT_dr [P, 4, 2, S]: head h=3g+q at partitions 32q:32q+32, group
    g, plane pl <-> d=2r+pl. (Bases are restricted to {0,32,64} - PE
    quadrant 3 is unusable - so 3 heads per group, partitions 96:128
    idle.) fp8 q/k is the one lossy step: its error enters pre-softmax
    and is damped by the small score variance (~1.4e-2 of the 2e-2
    budget); everything else stays bf16-exact.
  - Scores as fp8 DoubleRow (0.5 cycles/row, 2 d-planes per instruction):
    zT[k,q] per head from KT_dr/QT_dr [32,2,*] slices; head pairs at
    disjoint 32-row bases so real hardware row-tiles them concurrently.
  - exp on ACT with scale=(1/8)/4096 fused (absorbs the 64*64 q/k
    prescale), output bf16 at [k, q] tiles - 192 exps of [128,1024] are
    the ACT floor (~200us) that bounds the kernel.
  - PV with at as the STATIONARY operand: per (head, q-tile), 8
    accumulating matmuls lhsT=at[:,qt*128:+128], rhs=V[:,kt,h,0:65]
    (moving free dim = 65, so the whole PV pass is ~50k cycles). V
    carries a ones column, so the softmax denominator lands per-partition
    in PSUM column 64 - the normalization becomes one tiny reciprocal
    [128,8] plus one broadcast-AP multiply per head. No partition
    broadcasts, no gpsimd shifts.
  - O [q, hd] transposes back to OTn [hd, q] via bf16 PE transposes (odd
    heads straight to partitions 64:128), then the output projection runs
    in bf16 over chunk pairs exactly like the baseline.
  - Weight loads/conversions hoisted out of the batch loop.
"""

import numpy as np
from contextlib import ExitStack

import concourse.bacc as bacc
import concourse.bass as bass
import concourse.tile as tile
import concourse.mybir as mybir
from concourse.bass_utils import run_bass_kernel_spmd
from concourse.masks import make_identity

B, S, D, H, HD = 16, 1024, 768, 12, 64
NCORES = 8
BL = B // NCORES      # batches per core
P = 128
DC = D // P           # 6 contraction chunks
SQ = S // P           # 8 seq tiles of 128
F32 = mybir.dt.float32
F32R = mybir.dt.float32r
BF16 = mybir.dt.bfloat16
F8 = mybir.dt.float8e4
DR = mybir.MatmulPerfMode.DoubleRow
EXP = mybir.ActivationFunctionType.Exp
SCALE = 1.0 / float(np.sqrt(HD))
QS = 64.0                      # q/k fp8 pre-scale
ESCALE = SCALE / (QS * QS)     # exp scale absorbing the q/k scaling

_NC = {}
_DEBUG = False  # add DRAM dumps of intermediates (batch 0)


def _emit(tc, x_d, w_d, b_d, y_d, dbg=None, with_bias=True):
    """Emit the whole per-core program. w_d/b_d: dicts of DRAM APs."""
    nc = tc.nc

    def dump(name, sbuf_ap):
        if dbg is not None and name in dbg:
            nc.sync.dma_start(out=dbg[name], in_=sbuf_ap)

    with ExitStack() as ctx:
        consts = ctx.enter_context(tc.tile_pool(name="consts", bufs=1))
        wpool = ctx.enter_context(tc.tile_pool(name="wpool", bufs=1))
        big = ctx.enter_context(tc.tile_pool(name="big", bufs=2))
        atp = ctx.enter_context(tc.tile_pool(name="atp", bufs=20))
        iop = ctx.enter_context(tc.tile_pool(name="iop", bufs=2))
        smal = ctx.enter_context(tc.tile_pool(name="smal", bufs=4))
        pp = ctx.enter_context(tc.tile_pool(name="pp", bufs=2, space="PSUM"))

        # ---- constants ----
        ident = consts.tile([P, P], F32)
        make_identity(nc, ident)
        identb = consts.tile([P, P], BF16)
        nc.vector.tensor_copy(identb, ident)
        if with_bias:
            bq_sb = consts.tile([P, DC], F32)
            nc.sync.dma_start(out=bq_sb,
                              in_=b_d["bq"].rearrange("(c p) -> p c", p=P))
            nc.vector.tensor_scalar_mul(bq_sb, bq_sb, QS)
            bk_sb = consts.tile([P, DC], F32)
            nc.sync.dma_start(out=bk_sb,
                              in_=b_d["bk"].rearrange("(c p) -> p c", p=P))
            nc.vector.tensor_scalar_mul(bk_sb, bk_sb, QS)
            bv_st = consts.tile([P, DC], F32)
            nc.sync.dma_start(out=bv_st,
                              in_=b_d["bv"].rearrange("(c p) -> p c", p=P))
            bv_r = consts.tile([P, DC], BF16)
            nc.vector.tensor_copy(bv_r, bv_st)
            bo_st = consts.tile([1, D], F32)
            nc.sync.dma_start(out=bo_st, in_=b_d["bo"].unsqueeze(0))
            bo_r = consts.tile([1, D], BF16)
            nc.vector.tensor_copy(bo_r, bo_st)
            ones_f32 = consts.tile([1, P], F32)
            nc.vector.memset(ones_f32, 1.0)
            ones_row_r = consts.tile([1, P], BF16)
            nc.vector.tensor_copy(ones_row_r, ones_f32)
            cvec_sb = consts.tile([1, D], BF16)
        # warm the ACT exp table at t=0 so the ~1.3us table load overlaps
        # phase A instead of stalling the first attention tile
        expwarm = consts.tile([1, 1], F32)
        nc.scalar.activation(expwarm, ident[0:1, 0:1], EXP)
        cvec_state = {"done": False}

        # ---- weights: load once, convert to bf16 ----
        def load_weight(name):
            wr = wpool.tile([P, DC, D], BF16, tag=f"w_{name}", name=f"w_{name}")
            src = w_d[name].rearrange("(c p) m -> p c m", p=P)
            for c in range(0, DC, 2):
                ws = iop.tile([P, 2, D], F32, tag="st2", name=f"ws_{name}_{c}")
                nc.sync.dma_start(out=ws, in_=src[:, c:c + 2, :])
                nc.vector.tensor_copy(wr[:, c:c + 2, :], ws)
            return wr

        weights = {}

        def get_weight(name):
            if name not in weights:
                weights[name] = load_weight(name)
            return weights[name]

        def emit_cvec():
            wo_r = get_weight("wo")
            cv = pp.tile([P, 512], F32, tag="mm", name="cvps")
            cv2 = pp.tile([P, 256], F32, tag="mm", name="cvps2")
            for c in range(DC):
                nc.tensor.matmul(cv[0:1, :], bv_r[:, c:c + 1],
                                 wo_r[:, c, 0:512], start=(c == 0),
                                 stop=False)
                nc.tensor.matmul(cv2[0:1, :], bv_r[:, c:c + 1],
                                 wo_r[:, c, 512:D], start=(c == 0),
                                 stop=False)
            nc.tensor.matmul(cv[0:1, :], ones_row_r[:, 0:1],
                             bo_r[:, 0:512], start=False, stop=True)
            nc.tensor.matmul(cv2[0:1, :], ones_row_r[:, 0:1],
                             bo_r[:, 512:D], start=False, stop=True)
            nc.vector.tensor_copy(cvec_sb[:, 0:512], cv[0:1, :])
            nc.vector.tensor_copy(cvec_sb[:, 512:D], cv2[0:1, :])

        def phase_D(b, OTn, y_b):
            wo_r = get_weight("wo")
            if with_bias and not cvec_state["done"]:
                cvec_state["done"] = True
                emit_cvec()
            for sq in range(0, SQ, 2):
                yst = iop.tile([P, 2, D], F32, tag="st2", name=f"yst_{b}_{sq}")
                # split the final store so its first half ships while the
                # last tile is still evicting (shorter kernel tail)
                split = (b == BL - 1 and sq == SQ - 2)
                for j in range(2):
                    y0 = pp.tile([P, 512], F32, tag="mm",
                                 name=f"y0_{b}_{sq}_{j}")
                    y1 = pp.tile([P, 256], F32, tag="mm",
                                 name=f"y1_{b}_{sq}_{j}")
                    for c in range(DC):
                        st = OTn[:, c, (sq + j) * P:(sq + j + 1) * P]
                        last = (not with_bias) and c == DC - 1
                        nc.tensor.matmul(y0, st, wo_r[:, c, 0:512],
                                         start=(c == 0), stop=last)
                        nc.tensor.matmul(y1, st, wo_r[:, c, 512:D],
                                         start=(c == 0), stop=last)
                    if with_bias:
                        nc.tensor.matmul(y0, ones_row_r, cvec_sb[:, 0:512],
                                         start=False, stop=True)
                        nc.tensor.matmul(y1, ones_row_r, cvec_sb[:, 512:D],
                                         start=False, stop=True)
                    nc.vector.tensor_copy(yst[:, j, 0:512], y0)
                    if split and j == 1:
                        nc.sync.dma_start(out=y_b[:, sq + j, 0:512],
                                          in_=yst[:, j, 0:512])
                    nc.vector.tensor_copy(yst[:, j, 512:D], y1)
                    if split and j == 1:
                        nc.sync.dma_start(out=y_b[:, sq + j, 512:D],
                                          in_=yst[:, j, 512:D])
                    elif split:
                        nc.sync.dma_start(out=y_b[:, sq + j, :],
                                          in_=yst[:, j, :])
                if not split:
                    nc.sync.dma_start(out=y_b[:, sq:sq + 2, :], in_=yst)

        pending_D = None
        for b in range(BL):
            x_b = x_d[b].rearrange("(t p) d -> p t d", p=P)
            y_b = y_d[b].rearrange("(t p) d -> p t d", p=P)

            # ---- phase A: x -> xT (bf16) ----
            xT = big.tile([P, DC, S], BF16, tag="xT", name=f"xT_{b}")
            for sq in range(0, SQ, 2):
                x_in = iop.tile([P, 2, D], F32, tag="st2", name=f"xin_{b}_{sq}")
                if b == 0 and sq == 0:
                    # split the first load so the first transposes start as
                    # soon as the first columns land (shorter kernel lead-in)
                    nc.sync.dma_start(out=x_in[:, 0, 0:384],
                                      in_=x_b[:, sq, 0:384])
                    nc.sync.dma_start(out=x_in[:, 0, 384:D],
                                      in_=x_b[:, sq, 384:D])
                    nc.sync.dma_start(out=x_in[:, 1, :], in_=x_b[:, sq + 1, :])
                else:
                    nc.sync.dma_start(out=x_in, in_=x_b[:, sq:sq + 2, :])
                for j in range(2):
                    scol = slice((sq + j) * P, (sq + j + 1) * P)
                    tt0 = pp.tile([P, 512], F32, tag="mm",
                                  name=f"t0_{b}_{sq}_{j}")
                    for c in range(4):
                        nc.tensor.transpose(
                            tt0[:, c * P:(c + 1) * P],
                            x_in[:, j, c * P:(c + 1) * P], ident)
                    nc.vector.tensor_copy(
                        xT[:, 0:4, scol],
                        tt0.rearrange("p (c q) -> p c q", c=4))
                    tt1 = pp.tile([P, 256], F32, tag="mm",
                                  name=f"t1_{b}_{sq}_{j}")
                    for c in range(2):
                        nc.tensor.transpose(
                            tt1[:, c * P:(c + 1) * P],
                            x_in[:, j, (4 + c) * P:(5 + c) * P], ident)
                    nc.vector.tensor_copy(
                        xT[:, 4:6, scol],
                        tt1.rearrange("p (c q) -> p c q", c=2))

            if b == 0:
                dump("xT", xT)

            # ---- phase B: projections (bf16) ----
            # Q/K: accumulate per natural m-block/half, evict *64 to fp8
            # staging, then shuffle-DMA into the quad layout.
            QT = big.tile([P, 4, 2, S], F8, tag="QT", name=f"QT_{b}")
            KT = big.tile([P, 4, 2, S], F8, tag="KT", name=f"KT_{b}")
            stgs = {"q": big.tile([P, DC, S], F8, tag="qstg",
                                  name=f"qstg_{b}", bufs=1),
                    "k": big.tile([P, DC, S], F8, tag="kstg",
                                  name=f"kstg_{b}", bufs=1)}
            for mb in range(DC):
                for (wr, bname, qs, qdr) in (
                        (get_weight("wq"), "bq", "q", QT),
                        (get_weight("wk"), "bk", "k", KT)):
                    stg = stgs[qs]
                    for hf in range(2):
                        qq = pp.tile([P, 512], F32, tag="mm",
                                     name=f"{qs}ps_{b}_{mb}_{hf}")
                        for c in range(DC):
                            nc.tensor.matmul(
                                qq, wr[:, c, mb * P:(mb + 1) * P],
                                xT[:, c, hf * 512:(hf + 1) * 512],
                                start=(c == 0), stop=(c == DC - 1))
                        dst = stg[:, mb, hf * 512:(hf + 1) * 512]
                        if with_bias:
                            bias = bq_sb if bname == "bq" else bk_sb
                            nc.vector.tensor_scalar(
                                dst, qq, QS, bias[:, mb:mb + 1],
                                mybir.AluOpType.mult, mybir.AluOpType.add)
                        else:
                            nc.vector.tensor_scalar_mul(dst, qq, QS)
                    # shuffle natural block mb (heads 2mb,2mb+1; row 64j+d)
                    # into quad: head h at partitions 32*(h%3)+r, group
                    # h//3, plane pl=d%2, with d=2r+pl.
                    for j in range(2):
                        h = 2 * mb + j
                        g, qb = h // 3, 32 * (h % 3)
                        dsl = stg[64 * j:64 * (j + 1), mb, :]
                        # one DMA per plane: a partition-crossing middle
                        # dim ([pitch, 2]) is silently dropped by the DGE,
                        # but a strided partition dim ([2*pitch, 32]) works
                        for pl in range(2):
                            src = bass.AP(
                                tensor=dsl.tensor,
                                offset=dsl.offset + pl * dsl.ap[0][0],
                                ap=[[2 * dsl.ap[0][0], 32],
                                    list(dsl.ap[1])])
                            nc.gpsimd.dma_start(
                                out=qdr[qb:qb + 32, g, pl, :], in_=src)

            # V layout [P, SQ, H, 65] bf16: cols 0..63 = v, col 64 = ones
            V = big.tile([P, SQ, H, 65], BF16, tag="V", name=f"V_{b}")
            nc.vector.memset(V[:, :, :, 64], 1.0)
            wv_r = get_weight("wv")
            for sq in range(SQ):
                vv0 = pp.tile([P, 512], F32, tag="mm", name=f"v0_{b}_{sq}")
                vv1 = pp.tile([P, 256], F32, tag="mm", name=f"v1_{b}_{sq}")
                for c in range(DC):
                    nc.tensor.matmul(
                        vv0, xT[:, c, sq * P:(sq + 1) * P],
                        wv_r[:, c, 0:512], start=(c == 0), stop=(c == DC - 1))
                    nc.tensor.matmul(
                        vv1, xT[:, c, sq * P:(sq + 1) * P],
                        wv_r[:, c, 512:D], start=(c == 0), stop=(c == DC - 1))
                nc.vector.tensor_copy(
                    V[:, sq, 0:8, 0:64],
                    vv0.rearrange("p (h e) -> p h e", h=8))
                nc.vector.tensor_copy(
                    V[:, sq, 8:12, 0:64],
                    vv1.rearrange("p (h e) -> p h e", h=4))
            if b == 0:
                dump("QT", QT)
                dump("KT", KT)
                dump("V", V)

            # pipeline: previous batch's output projection goes here so its
            # psum-ring slots sit AFTER this batch's A/B tiles (overlap with
            # this batch's phase C).
            if pending_D is not None:
                phase_D(*pending_D)

            # ---- phase C: attention ----
            # scores: fp8 DoubleRow, head pairs at disjoint row bases.
            # PV: at-stationary, V-moving, q-tiles in two halves;
            # denominator accumulates in psum col 64 per q-tile.
            OTn = big.tile([P, DC, S], BF16, tag="OTn", name=f"OTn_{b}",
                           bufs=1)
            for ch in range(DC):
                hs = (2 * ch, 2 * ch + 1)
                ooh0 = [pp.tile([P, 4, 65], F32, tag="ov",
                                name=f"o0_{b}_{ch}_{par}")
                        for par in range(2)]
                ats = []
                for kt in range(SQ):
                    zzs = [pp.tile([P, 1024], F32, tag="zz",
                                   name=f"zps_{b}_{ch}_{par}_{kt}")
                           for par in range(2)]
                    for hf in range(2):
                        for par in range(2):
                            h = hs[par]
                            g, base = h // 3, 32 * (h % 3)
                            ksl = KT[base:base + 32, g, :,
                                     kt * P:(kt + 1) * P]
                            qsl = QT[base:base + 32, g, :,
                                     hf * 512:(hf + 1) * 512]
                            nc.tensor.matmul(
                                zzs[par][:, hf * 512:(hf + 1) * 512],
                                ksl, qsl, start=True, stop=True,
                                perf_mode=DR)
                    row = []
                    for par in range(2):
                        at = atp.tile([P, S], BF16, tag="at",
                                      name=f"at_{b}_{ch}_{par}_{kt}")
                        nc.scalar.activation(at, zzs[par], EXP, scale=ESCALE)
                        row.append(at)
                        if b == 0 and ch == 0 and par == 0 and kt == 0:
                            dump("at0", at)
                    ats.append(row)
                    for par in range(2):
                        vsl = V[:, kt, hs[par], :]
                        for qt in range(4):
                            # one start per psum bank: start=True lazily
                            # zeroes the whole 2KB region, so only the
                            # tile's first matmul may carry it
                            nc.tensor.matmul(
                                ooh0[par][:, qt, :],
                                row[par][:, qt * P:(qt + 1) * P], vsl,
                                start=(kt == 0 and qt == 0),
                                stop=(kt == SQ - 1),
                                skip_group_check=True)
                # normalize half 0 early to free its psum slots
                osbs = {}

                def normalize(ooh, half, ch=ch):
                    for par in range(2):
                        h = hs[par]
                        rb = smal.tile([P, 4], F32, tag="rb",
                                       name=f"rb_{b}_{h}_{half}")
                        nc.vector.reciprocal_approx_fast(
                            out=rb, in_=ooh[par][:, :, 64])
                        osb = smal.tile([P, 4, HD], BF16, tag="osb",
                                        name=f"osb_{b}_{h}_{half}")
                        rbb = bass.AP(tensor=rb.tensor, offset=rb.offset,
                                      ap=[list(rb.ap[0]), list(rb.ap[1]),
                                          [0, HD]])
                        nc.vector.tensor_mul(osb, ooh[par][:, :, 0:64], rbb)
                        osbs[(par, half)] = osb

                normalize(ooh0, 0)
                ooh1 = [pp.tile([P, 4, 65], F32, tag="ov",
                                name=f"o1_{b}_{ch}_{par}")
                        for par in range(2)]
                for kt in range(SQ):
                    for par in range(2):
                        vsl = V[:, kt, hs[par], :]
                        for qt in range(4, SQ):
                            nc.tensor.matmul(
                                ooh1[par][:, qt - 4, :],
                                ats[kt][par][:, qt * P:(qt + 1) * P], vsl,
                                start=(kt == 0 and qt == 4),
                                stop=(kt == SQ - 1),
                                skip_group_check=True)
                normalize(ooh1, 1)
                # transpose O [q, hd] -> OTn [hd, q] (odd head straight to
                # rows 64:128)
                tps = pp.tile([P, 1024], BF16, tag="zz",
                              name=f"tops_{b}_{ch}")
                for par in range(2):
                    for qt in range(SQ):
                        nc.tensor.transpose(
                            tps[64 * par:64 * (par + 1),
                                qt * P:(qt + 1) * P],
                            osbs[(par, qt // 4)][:, qt % 4, :], identb)
                nc.vector.tensor_copy(OTn[:, ch, :], tps)
            if b == 0:
                dump("OTn", OTn)

            pending_D = (b, OTn, y_b)

        phase_D(*pending_D)


def _build(with_bias=True):
    nc = bacc.Bacc("TRN2", target_bir_lowering=False, debug=False,
                   num_devices=NCORES)
    x_d = nc.dram_tensor("x", [BL, S, D], F32, kind="ExternalInput").ap()
    w_d = {n: nc.dram_tensor(n, [D, D], F32, kind="ExternalInput").ap()
           for n in ("wq", "wk", "wv", "wo")}
    b_d = {n: nc.dram_tensor(n, [D], F32, kind="ExternalInput").ap()
           for n in ("bq", "bk", "bv", "bo")}
    y_d = nc.dram_tensor("y", [BL, S, D], F32, kind="ExternalOutput").ap()
    dbg = None
    if _DEBUG:
        shapes = {"xT": ([P, DC, S], BF16), "QT": ([P, 4, 2, S], F8),
                  "KT": ([P, 4, 2, S], F8), "V": ([P, SQ, H, 65], BF16),
                  "at0": ([P, S], BF16), "rbi0": ([P, 4], F32),
                  "osb0": ([P, 4, HD], BF16), "OTn": ([P, DC, S], BF16)}
        dbg = {n: nc.dram_tensor(f"dbg_{n}", sh, dt,
                                 kind="ExternalOutput").ap()
               for n, (sh, dt) in shapes.items()}
    with tile.TileContext(nc) as tc:
        _emit(tc, x_d, w_d, b_d, y_d, dbg, with_bias=with_bias)
    nc.compile()
    return nc


def _in_maps(x, Wq, bq, Wk, bk, Wv, bv, Wo, bo):
    # convert to host numpy before reshaping so jax-array inputs don't
    # trigger device-side ops
    def _np(a, shape):
        return np.ascontiguousarray(
            np.asarray(a, dtype=np.float32).reshape(shape))

    w = {
        "wq": _np(Wq, (D, D)), "wk": _np(Wk, (D, D)),
        "wv": _np(Wv, (D, D)), "wo": _np(Wo, (D, D)),
        "bq": _np(bq, (D,)), "bk": _np(bk, (D,)),
        "bv": _np(bv, (D,)), "bo": _np(bo, (D,)),
    }
    x = np.asarray(x, dtype=np.float32)
    return [dict(w, x=np.ascontiguousarray(x[i * BL:(i + 1) * BL]))
            for i in range(NCORES)]


def get_nc(with_bias=True):
    if with_bias not in _NC:
        _NC[with_bias] = _build(with_bias=with_bias)
    return _NC[with_bias]


def run(inputs, trace=False):
    with_bias = any(
        np.any(np.asarray(inputs[k])) for k in ("bq", "bk", "bv", "bo"))
    nc = get_nc(with_bias=with_bias)
    maps = _in_maps(**inputs)
    res = run_bass_kernel_spmd(nc, maps, list(range(NCORES)), trace=trace)
    y = np.concatenate([res.results[i]["y"] for i in range(NCORES)], axis=0)
    return y, res


def kernel(x, Wq, bq, Wk, bk, Wv, bv, Wo, bo):
    y, _ = run(dict(x=x, Wq=Wq, bq=bq, Wk=Wk, bk=bk, Wv=Wv, bv=bv,
                    Wo=Wo, bo=bo))
    return y
